# revision 11
# baseline (speedup 1.0000x reference)
import sys

sys.path.insert(0, "/opt/trn_rl_repo")
import numpy as np

import concourse.bass as bass
import concourse.tile as tile
from concourse import bacc, mybir
from concourse.bass_utils import run_bass_kernel_spmd
from concourse.masks import make_identity

f32 = mybir.dt.float32
bf16 = mybir.dt.bfloat16
fp16 = mybir.dt.float16
u32 = mybir.dt.uint32
Exp = mybir.ActivationFunctionType.Exp
AX = mybir.AxisListType.X
MAX = mybir.AluOpType.max

B, N, D = 4, 4096, 64
NCORES = 8
NQ = 2048  # queries per core (half a batch)
NK = 4096  # keys per core
QT = NQ // 128  # 16 q-tiles
CH = NK // 128  # 32 k-chunks
BLK = 4  # query blocks of 512
SCALE = 64.0  # sqrt(N)

_cached = {}


def build_program():
    nc = bacc.Bacc("TRN2", target_bir_lowering=False, debug=False, num_devices=NCORES)
    q_d = nc.dram_tensor("q", [NQ, D], f32, kind="ExternalInput").ap()
    k_d = nc.dram_tensor("k", [NK, D], f32, kind="ExternalInput").ap()
    v_d = nc.dram_tensor("v", [NK, D], f32, kind="ExternalInput").ap()
    o_d = nc.dram_tensor("o", [NQ, D], f32, kind="ExternalOutput").ap()
    # partition-major query/key layout: partition p holds queries {16p+t},
    # keys {32p+c}; the output DMA applies the inverse permutation.
    q3 = q_d.rearrange("(p t) d -> p t d", p=128)
    k3 = k_d.rearrange("(p c) d -> p c d", p=128)
    v3 = v_d.rearrange("(p c) d -> p c d", p=128)
    o3 = o_d.rearrange("(p t) d -> p t d", p=128)

    with tile.TileContext(nc) as tc:
        import contextlib

        ctx = contextlib.ExitStack()
        with ctx:
            big = ctx.enter_context(tc.tile_pool(name="big", bufs=1))
            atp = ctx.enter_context(tc.tile_pool(name="atp", bufs=3))
            qk_ps = ctx.enter_context(tc.tile_pool(name="qk_ps", bufs=2, space="PSUM"))
            mp_ps = ctx.enter_context(tc.tile_pool(name="mp_ps", bufs=1, space="PSUM"))
            pv_ps = ctx.enter_context(tc.tile_pool(name="pv_ps", bufs=1, space="PSUM"))

            ident16 = big.tile([128, 128], fp16)
            make_identity(nc, ident16[:])
            ident32 = big.tile([128, 128], f32)
            make_identity(nc, ident32[:])

            q_sb = big.tile([128, QT, D], f32)
            k_sb = big.tile([128, CH, D], f32)
            v_sb = big.tile([128, CH, D], f32)
            q16 = big.tile([128, QT, D], fp16)
            k16 = big.tile([128, CH, D], fp16)
            v_aug = big.tile([128, CH, 65], bf16)
            kT_pack = big.tile([65, CH, 128], fp16)
            qkmov = big.tile([65, QT, 128], fp16)
            M_all = big.tile([128, QT], f32)
            mpp = big.tile([128, QT, 3], f32)  # per-fill max partials
            out_all = big.tile([128, QT, D], f32)
            rZ = big.tile([128, BLK * 4], f32)

            # ---- input DMAs (contiguous per-partition descriptors)
            nc.sync.dma_start(out=q_sb[:], in_=q3)
            nc.sync.dma_start(out=k_sb[:], in_=k3)
            nc.sync.dma_start(out=v_sb[:], in_=v3)

            # ---- gpsimd casts (f32 -> fp16/bf16) + constants
            nc.gpsimd.tensor_copy(q16[:], q_sb[:])
            nc.gpsimd.tensor_copy(k16[:], k_sb[:])
            nc.gpsimd.tensor_copy(v_aug[:, :, 0:64], v_sb[:])
            nc.gpsimd.memset(v_aug[:, :, 64:65], 1.0)
            # row 64 of kT_pack = -1.0 in fp16 (0xBC00 pairs packed in u32)
            nc.gpsimd.memset(kT_pack[64:65, :, :].bitcast(u32), 0xBC00BC00)

            # ---- PE transposes to build kT_pack rows 0:64 and qkmov rows 0:64
            def transpose_batch(src, n_tiles, dst, dst_off):
                # src: [128, n, 64] fp16; dst rows 0:64, tiles dst_off..
                for g in range((n_tiles + 3) // 4):
                    p_t = mp_ps.tile([128, 3, 512], f32, tag="mp")
                    p16 = p_t[:, 0, :].bitcast(fp16)  # [128, 1024] fp16 view
                    cnt = min(4, n_tiles - g * 4)
                    for i in range(cnt):
                        c = g * 4 + i
                        nc.tensor.transpose(
                            p16[0:64, i * 128 : (i + 1) * 128],
                            src[:, c, :],
                            ident16[:],
                        )
                    nc.scalar.copy(
                        dst[0:64, dst_off + g * 4 : dst_off + g * 4 + cnt, :],
                        p16[0:64, 0 : cnt * 128].rearrange(
                            "p (c x) -> p c x", x=128
                        ),
                    )

            transpose_batch(k16, CH, kT_pack, 0)
            transpose_batch(q16, QT, qkmov, 0)

            # ---- stage generators -------------------------------------
            def mp_stage(b):
                # max-pass for q-tiles 4b..4b+3: scores [128q, 512k] per mm
                # into a wide psum tile, reduced 1536-at-a-time on DVE
                for ti in range(4):
                    t = 4 * b + ti
                    for fi, nmm in enumerate((3, 3, 2)):
                        p_m = mp_ps.tile([128, 3, 512], f32, tag="mp")
                        for i in range(nmm):
                            g = fi * 3 + i
                            nc.tensor.matmul(
                                p_m[:, i, :],
                                qkmov[0:64, t, :],
                                kT_pack[0:64, 4 * g : 4 * g + 4, :],
                                start=True,
                                stop=True,
                            )
                        nc.vector.reduce_max(
                            mpp[:, t, fi : fi + 1],
                            p_m[:, 0:nmm, :],
                            axis=mybir.AxisListType.XY,
                        )
                        yield
                    nc.vector.reduce_max(M_all[:, t : t + 1], mpp[:, t, :], axis=AX)

            def mhat_stage(b):
                # transpose M into row layout, write qkmov row 64
                mt_t = mp_ps.tile([128, 3, 512], f32, tag="mp")
                for j in range(4):
                    nc.tensor.transpose(
                        mt_t[0:1, 0, j * 128 : (j + 1) * 128],
                        M_all[:, 4 * b + j : 4 * b + j + 1],
                        ident32[:],
                    )
                for j in range(4):
                    nc.vector.tensor_copy(
                        qkmov[64:65, 4 * b + j, :],
                        mt_t[0:1, 0, j * 128 : (j + 1) * 128],
                    )

            def block_stage(b):
                # QK (fp16, with -M row) -> exp -> PV. PV for group g is
                # emitted after QK of group g+1 so the PE never waits on ACT.
                p_o = pv_ps.tile([128, 4, 65], f32, tag="pv")

                def pv_emit(g, at):
                    for h in range(2):
                        c = g * 2 + h
                        for j in range(4):
                            nc.tensor.matmul(
                                p_o[:, j, :],
                                at[:, h, j * 128 : (j + 1) * 128],
                                v_aug[:, c, :],
                                start=(c == 0 and j == 0),
                                stop=(c == CH - 1 and j == 3),
                            )

                pending = None
                for g in range(CH // 2):
                    p_s = qk_ps.tile([128, 1024], f32, tag="qk")
                    for h in range(2):
                        c = g * 2 + h
                        nc.tensor.matmul(
                            p_s[:, h * 512 : (h + 1) * 512],
                            kT_pack[:, c, :],
                            qkmov[:, 4 * b : 4 * b + 4, :],
                            start=True,
                            stop=True,
                        )
                    at = atp.tile([128, 2, 512], bf16, tag="at")
                    nc.scalar.activation(
                        out=at[:], in_=p_s[:], func=Exp, bias=0.0, scale=SCALE
                    )
                    if pending is not None:
                        pv_emit(*pending)
                    pending = (g, at)
                    yield
                pv_emit(*pending)
                # epilogue: normalize by Z (column 64) and store
                for j in range(4):
                    r = rZ[:, 4 * b + j : 4 * b + j + 1]
                    nc.vector.reciprocal(r, p_o[:, j, 64:65])
                    nc.vector.tensor_scalar_mul(
                        out_all[:, 4 * b + j, :], p_o[:, j, 0:64], r
                    )
                nc.sync.dma_start(
                    out=o3[:, 4 * b : 4 * b + 4, :],
                    in_=out_all[:, 4 * b : 4 * b + 4, :],
                )
                yield

            def run_interleaved(gens):
                alive = list(gens)
                while alive:
                    for gen in list(alive):
                        try:
                            next(gen)
                        except StopIteration:
                            alive.remove(gen)

            # software pipeline: mp(0); then [mp(b+1) | block(b)] interleaved
            for _ in mp_stage(0):
                pass
            mhat_stage(0)
            for b in range(BLK):
                gens = [block_stage(b)]
                if b + 1 < BLK:
                    gens.append(mp_stage(b + 1))
                run_interleaved(gens)
                if b + 1 < BLK:
                    mhat_stage(b + 1)

    nc.compile()
    return nc


def kernel(q, k, v):
    if "nc" not in _cached:
        _cached["nc"] = build_program()
    nc = _cached["nc"]
    in_maps = []
    for c in range(NCORES):
        b, h = c // 2, c % 2
        in_maps.append(
            {
                "q": np.ascontiguousarray(q[b, h * NQ : (h + 1) * NQ, :]),
                "k": np.ascontiguousarray(k[b]),
                "v": np.ascontiguousarray(v[b]),
            }
        )
    res = run_bass_kernel_spmd(nc, in_maps, list(range(NCORES)))
    out = np.empty((B, N, D), dtype=np.float32)
    for c in range(NCORES):
        b, h = c // 2, c % 2
        out[b, h * NQ : (h + 1) * NQ, :] = res.results[c]["o"]
    return out


# revision 19
# speedup vs baseline: 1.3431x; 1.3431x over previous
import sys

sys.path.insert(0, "/opt/trn_rl_repo")
import numpy as np

import concourse.bass as bass
import concourse.tile as tile
from concourse import bacc, mybir
from concourse.bass_utils import run_bass_kernel_spmd
from concourse.masks import make_identity

f32 = mybir.dt.float32
bf16 = mybir.dt.bfloat16
fp16 = mybir.dt.float16
u32 = mybir.dt.uint32
Exp = mybir.ActivationFunctionType.Exp
AX = mybir.AxisListType.X
MAX = mybir.AluOpType.max

B, N, D = 4, 4096, 64
NCORES = 8
NQ = 2048  # queries per core (half a batch)
NK = 4096  # keys per core
QT = NQ // 128  # 16 q-tiles
CH = NK // 128  # 32 k-chunks
BLK = 4  # query blocks of 512 (legacy)
BLOCKS = [(2 * i, 2) for i in range(8)]  # (tile0, ntiles)
SCALE = 64.0  # sqrt(N)

_cached = {}


def build_program():
    nc = bacc.Bacc("TRN2", target_bir_lowering=False, debug=False, num_devices=NCORES)
    q_d = nc.dram_tensor("q", [NQ, D], f32, kind="ExternalInput").ap()
    k_d = nc.dram_tensor("k", [NK, D], f32, kind="ExternalInput").ap()
    v_d = nc.dram_tensor("v", [NK, D], f32, kind="ExternalInput").ap()
    o_d = nc.dram_tensor("o", [NQ, D], f32, kind="ExternalOutput").ap()
    # partition-major query/key layout: partition p holds queries {16p+t},
    # keys {32p+c}; the output DMA applies the inverse permutation.
    q3 = q_d.rearrange("(p t) d -> p t d", p=128)
    k3 = k_d.rearrange("(p c) d -> p c d", p=128)
    v3 = v_d.rearrange("(p c) d -> p c d", p=128)
    o3 = o_d.rearrange("(p t) d -> p t d", p=128)

    with tile.TileContext(nc) as tc:
        import contextlib

        ctx = contextlib.ExitStack()
        with ctx:
            big = ctx.enter_context(tc.tile_pool(name="big", bufs=1))
            atp = ctx.enter_context(tc.tile_pool(name="atp", bufs=3))
            qk_ps = ctx.enter_context(tc.tile_pool(name="qk_ps", bufs=2, space="PSUM"))
            mp_ps = ctx.enter_context(tc.tile_pool(name="mp_ps", bufs=1, space="PSUM"))
            mpb_ps = ctx.enter_context(tc.tile_pool(name="mpb_ps", bufs=1, space="PSUM"))
            pv_ps = ctx.enter_context(tc.tile_pool(name="pv_ps", bufs=1, space="PSUM"))

            ident16 = big.tile([128, 128], fp16)
            make_identity(nc, ident16[:])
            ident32 = big.tile([128, 128], f32)
            make_identity(nc, ident32[:])

            q_sb = big.tile([128, QT, D], f32)
            k_sb = big.tile([128, CH, D], f32)
            v_sb = big.tile([128, CH, D], f32)
            q16 = big.tile([128, QT, D], fp16)
            k16 = big.tile([128, CH, D], fp16)
            v_aug = big.tile([128, CH, 65], bf16)
            kT_pack = big.tile([65, CH, 128], fp16)
            qkmov = big.tile([65, QT, 128], fp16)
            M_all = big.tile([128, QT], f32)
            mpp = big.tile([128, QT, 5], f32)  # per-fill max partials
            out_all = big.tile([128, QT, D], f32)
            rZ = big.tile([128, BLK * 4], f32)

            # ---- input DMAs (contiguous per-partition descriptors)
            nc.sync.dma_start(out=k_sb[:, 0:16, :], in_=k3[:, 0:16, :])
            nc.scalar.dma_start(out=q_sb[:], in_=q3)
            nc.sync.dma_start(out=k_sb[:, 16:32, :], in_=k3[:, 16:32, :])
            nc.scalar.dma_start(out=v_sb[:], in_=v3)

            # ---- casts (f32 -> fp16/bf16) + constants, split across engines
            nc.gpsimd.memset(kT_pack[64:65, :, :].bitcast(u32), 0xBC00BC00)
            nc.vector.tensor_copy(q16[:], q_sb[:])
            for half in range(2):
                nc.gpsimd.tensor_copy(
                    k16[:, half * 16 : (half + 1) * 16, :],
                    k_sb[:, half * 16 : (half + 1) * 16, :],
                )
            nc.vector.tensor_copy(v_aug[:, :, 0:64], v_sb[:])
            nc.vector.memset(v_aug[:, :, 64:65], 1.0)

            # ---- PE transposes to build kT_pack rows 0:64 and qkmov rows 0:64
            def transpose_batch(src, n_tiles, dst, dst_off):
                # src: [128, n, 64] fp16; dst rows 0:64, tiles dst_off..
                for g in range((n_tiles + 3) // 4):
                    p_t = mp_ps.tile([128, 2, 512], f32, tag="mp")
                    p16 = p_t[:, 0, :].bitcast(fp16)  # [128, 1024] fp16 view
                    cnt = min(4, n_tiles - g * 4)
                    for i in range(cnt):
                        c = g * 4 + i
                        nc.tensor.transpose(
                            p16[0:64, i * 128 : (i + 1) * 128],
                            src[:, c, :],
                            ident16[:],
                        )
                    nc.scalar.copy(
                        dst[0:64, dst_off + g * 4 : dst_off + g * 4 + cnt, :],
                        p16[0:64, 0 : cnt * 128].rearrange(
                            "p (c x) -> p c x", x=128
                        ),
                    )

            transpose_batch(k16, CH, kT_pack, 0)

            def qprep_gen():
                # q-tile batches emitted lazily so mp(0) interleaves
                for g in range(QT // 4):
                    transpose_batch(
                        q16[:, g * 4 : (g + 1) * 4, :], 4, qkmov, g * 4
                    )
                    yield

            _qp = qprep_gen()
            next(_qp)  # tiles 0-3 up front (mp(0)/mp(1) need them)

            def run_interleaved(gens):
                # gens: list of (gen, num, den): advance `num` steps every
                # `den` rounds
                state = [[g, num, den, 0] for g, num, den in gens]
                while state:
                    for ent in list(state):
                        gen, num, den, acc = ent
                        ent[3] = acc = acc + num
                        steps, ent[3] = divmod(acc, den)
                        for _ in range(steps):
                            try:
                                next(gen)
                            except StopIteration:
                                state.remove(ent)
                                break

            # ---- stage generators -------------------------------------
            def mp_stage(bi):
                t0, nt = BLOCKS[bi]
                # max-pass for q-tiles 4b..4b+3: scores [128q, 512k] per mm
                # into alternating psum tiles, reduced on DVE. The M-shuffle
                # for tile t is deferred a couple of fills so the PE transpose
                # never waits on the DVE reduce backlog.
                deferred = []

                def finish_tile(t):
                    nc.vector.reduce_max(M_all[:, t : t + 1], mpp[:, t, :], axis=AX)
                    mt_t = mpb_ps.tile([128, 1, 512], f32, tag="mpb")
                    nc.tensor.transpose(
                        mt_t[0:1, 0, 0:128],
                        M_all[:, t : t + 1],
                        ident32[:],
                    )
                    nc.scalar.copy(qkmov[64:65, t, :], mt_t[0:1, 0, 0:128])

                for ti in range(nt):
                    t = t0 + ti
                    g = 0
                    for fi, nmm in enumerate((2, 1, 2, 1, 2)):
                        if nmm == 2:
                            p_m = mp_ps.tile([128, 2, 512], f32, tag="mp")
                        else:
                            p_m = mpb_ps.tile([128, 1, 512], f32, tag="mpb")
                        for i in range(nmm):
                            nc.tensor.matmul(
                                p_m[:, i, :],
                                qkmov[0:64, t, :],
                                kT_pack[0:64, 4 * g : 4 * g + 4, :],
                                start=True,
                                stop=True,
                            )
                            g += 1
                        nc.vector.reduce_max(
                            mpp[:, t, fi : fi + 1],
                            p_m[:, 0:nmm, :],
                            axis=mybir.AxisListType.XY,
                        )
                        if fi == 2 and deferred:
                            finish_tile(deferred.pop(0))
                        yield
                    deferred.append(t)
                for t in deferred:
                    finish_tile(t)
                    yield

            def block_stage(bi):
                # QK (fp16, with -M row) -> exp -> PV. PV for group g is
                # emitted after QK of group g+1 so the PE never waits on ACT.
                t0, nt = BLOCKS[bi]
                cpt = 8 // nt  # chunks per exp tile (width 1024 cols)
                p_o = pv_ps.tile([128, nt, 65], f32, tag="pv")

                def pv_emit(g, at):
                    for cc in range(cpt):
                        c = g * cpt + cc
                        for j in range(nt):
                            nc.tensor.matmul(
                                p_o[:, j, :],
                                at[:, cc, j * 128 : (j + 1) * 128],
                                v_aug[:, c, :],
                                start=(c == 0 and j == 0),
                                stop=(c == CH - 1 and j == nt - 1),
                            )

                pending = None
                for g in range(CH // cpt):
                    p_s = qk_ps.tile([128, cpt, nt * 128], f32, tag="qk")
                    for cc in range(cpt):
                        c = g * cpt + cc
                        nc.tensor.matmul(
                            p_s[:, cc, :],
                            kT_pack[:, c, :],
                            qkmov[:, t0 : t0 + nt, :],
                            start=True,
                            stop=True,
                        )
                    at = atp.tile([128, cpt, nt * 128], bf16, tag="at")
                    nc.scalar.activation(
                        out=at[:], in_=p_s[:], func=Exp, bias=0.0, scale=SCALE
                    )
                    if pending is not None:
                        pv_emit(*pending)
                    pending = (g, at)
                    yield
                pv_emit(*pending)
                # epilogue: normalize by Z (column 64) and store
                for j in range(nt):
                    r = rZ[:, t0 + j : t0 + j + 1]
                    nc.vector.reciprocal(r, p_o[:, j, 64:65])
                    nc.scalar.mul(out_all[:, t0 + j, :], p_o[:, j, 0:64], r)
                nc.sync.dma_start(
                    out=o3[:, t0 : t0 + nt, :],
                    in_=out_all[:, t0 : t0 + nt, :],
                )
                yield

            # software pipeline: mp(0); then [mp(b+1) | block(b)] interleaved
            run_interleaved([(mp_stage(0), 2, 1), (_qp, 1, 1)])
            NB = len(BLOCKS)
            for bi in range(NB):
                _, nt = BLOCKS[bi]
                block_yields = CH // (8 // nt) + 1
                gens = [(block_stage(bi), 1, 1)]
                if bi + 1 < NB:
                    nt_next = BLOCKS[bi + 1][1]
                    mp_yields = 6 * nt_next
                    gens.append((mp_stage(bi + 1), mp_yields, block_yields))
                run_interleaved(gens)

    nc.compile()
    return nc


def kernel(q, k, v):
    if "nc" not in _cached:
        _cached["nc"] = build_program()
    nc = _cached["nc"]
    in_maps = []
    for c in range(NCORES):
        b, h = c // 2, c % 2
        in_maps.append(
            {
                "q": np.ascontiguousarray(q[b, h * NQ : (h + 1) * NQ, :]),
                "k": np.ascontiguousarray(k[b]),
                "v": np.ascontiguousarray(v[b]),
            }
        )
    res = run_bass_kernel_spmd(nc, in_maps, list(range(NCORES)))
    out = np.empty((B, N, D), dtype=np.float32)
    for c in range(NCORES):
        b, h = c // 2, c % 2
        out[b, h * NQ : (h + 1) * NQ, :] = res.results[c]["o"]
    return out
